# revision 34
# baseline (speedup 1.0000x reference)
"""Trainium2 Bass kernel for nn_MultiHeadAttention (B=2, S=2048, HID=2048, NH=16, HD=128).

Sharding: 8 cores = 2 batches x 4 head-groups (4 heads each). Each core computes
its head-group's attention context and a partial output projection (Megatron-TP
row-parallel Wo); host sums the 4 partials per batch and adds bo.

Per-core pipeline (fp16 PE datapath, fp32 PSUM accumulation), streamed over 4
sequence chunks of 512:
  K/Q/V projections (Q/K bias fused into PSUM->SBUF eviction on DVE; V bias
  applied host-side as bv @ Wo.T) -> scores (KQ^T per 128-k block, causally
  narrowed on diagonal chunks) -> exp on ACT -> causal mask on diagonal
  blocks (DVE multiply) -> PV matmuls with a ones-column in V giving the
  softmax denominator -> normalize (DVE reciprocal + tensor_scalar) -> PE
  transpose (deferred one step so PE overlaps the DVE chain) -> Wo partial
  matmul -> DMA out fp32 partial per 512-column block.
"""
import sys
sys.path.insert(0, "/opt/trn_rl_repo")

import math
import time
from contextlib import ExitStack

import numpy as np

import concourse.bass as bass  # noqa: F401  (registers AP machinery)
import concourse.bacc as bacc
import concourse.tile as tile
import concourse.masks as masks
from concourse import mybir
import concourse.bass2jax as b2j

HID, NH, HD = 2048, 16, 128
B, S = 2, 2048
CH = 4            # sequence chunks
CS = S // CH      # 512
KT16 = HID // 128  # 16 contraction tiles
FP16 = mybir.dt.float16
F32 = mybir.dt.float32

_NC = None
_RUNNER = None
LAST_DEVICE_NS = None
NCORES = 8


class _Runner:
    """Jit the bass_exec shard_map once; reuse across kernel() calls."""

    def __init__(self, nc):
        import jax
        import jax.numpy as jnp
        from jax.experimental.shard_map import shard_map
        from jax.sharding import Mesh, NamedSharding, PartitionSpec

        b2j.install_neuronx_cc_hook()
        partition_name = (
            nc.partition_id_tensor.name if nc.partition_id_tensor else None)
        in_names, out_names, out_avals, zero_specs = [], [], [], []
        for alloc in nc.m.functions[0].allocations:
            if not isinstance(alloc, mybir.MemoryLocationSet):
                continue
            name = alloc.memorylocations[0].name
            if alloc.kind == "ExternalInput":
                if name != partition_name:
                    in_names.append(name)
            elif alloc.kind == "ExternalOutput":
                shape = tuple(alloc.tensor_shape)
                dtype = mybir.dt.np(alloc.dtype)
                out_names.append(name)
                out_avals.append(jax.core.ShapedArray(shape, dtype))
                zero_specs.append((shape, dtype))
        n_params = len(in_names)
        n_outs = len(out_avals)
        all_in_names = list(in_names) + list(out_names)
        if partition_name is not None:
            all_in_names.append(partition_name)
        self.in_names = in_names
        self.out_names = out_names
        self.out_avals = out_avals

        def _body(*args):
            operands = list(args)
            if partition_name is not None:
                operands.append(b2j.partition_id_tensor())
            return tuple(b2j._bass_exec_p.bind(
                *operands,
                out_avals=tuple(out_avals),
                in_names=tuple(all_in_names),
                out_names=tuple(out_names),
                lowering_input_output_aliases=(),
                sim_require_finite=True,
                sim_require_nnan=True,
                nc=nc,
            ))

        devices = jax.devices()[:NCORES]
        assert len(devices) == NCORES
        mesh = Mesh(np.asarray(devices), ("core",))
        pspec = PartitionSpec("core")
        self.fn = jax.jit(
            shard_map(_body, mesh=mesh,
                      in_specs=(pspec,) * (n_params + n_outs),
                      out_specs=(pspec,) * n_outs, check_rep=False),
            donate_argnums=tuple(range(n_params, n_params + n_outs)),
            keep_unused=True,
        )
        shardings = tuple(NamedSharding(mesh, pspec) for _ in range(n_outs))
        self.in_sharding = NamedSharding(mesh, pspec)
        self.zeros_fn = jax.jit(
            lambda: tuple(
                jnp.zeros((NCORES * s[0], *s[1:]), d) for s, d in zero_specs),
            out_shardings=shardings,
        )
        self.jax = jax

    def __call__(self, in_maps, reps=6):
        jax = self.jax
        concat = [
            np.concatenate([np.asarray(m[name]) for m in in_maps], axis=0)
            for name in self.in_names
        ]
        dev_in = [jax.device_put(a, self.in_sharding) for a in concat]
        dev_in = jax.block_until_ready(dev_in)
        # warmup + output buffers for each timed rep (donated)
        zs = [self.zeros_fn() for _ in range(reps + 1)]
        zs = jax.block_until_ready(zs)
        outs = jax.block_until_ready(self.fn(*dev_in, *zs[0]))
        # timed, pipelined: dispatch all reps then block once
        t0 = time.time()
        all_outs = [self.fn(*dev_in, *zs[r + 1]) for r in range(reps)]
        jax.block_until_ready(all_outs)
        dt_ns = int((time.time() - t0) * 1e9 / reps)
        results = [
            {name: np.asarray(outs[i]).reshape(NCORES, *self.out_avals[i].shape)[c]
             for i, name in enumerate(self.out_names)}
            for c in range(NCORES)
        ]
        return results, dt_ns


def build_program(repeat=1):
    nc = bacc.Bacc(None, target_bir_lowering=False, debug=False)
    xt_d = nc.dram_tensor("xt", [CH, 128, KT16 * CS], FP16, kind="ExternalInput").ap()
    wq_d = nc.dram_tensor("wq", [128, 8192], FP16, kind="ExternalInput").ap()
    wk_d = nc.dram_tensor("wk", [128, 8192], FP16, kind="ExternalInput").ap()
    wv_d = nc.dram_tensor("wv", [128, 8192], FP16, kind="ExternalInput").ap()
    wo_d = nc.dram_tensor("wo", [128, 8192], FP16, kind="ExternalInput").ap()
    bqk_d = nc.dram_tensor("bqk", [128, 8], F32, kind="ExternalInput").ap()
    y_d = nc.dram_tensor("y", [16, 128, HID], F32, kind="ExternalOutput").ap()

    SCALE = 1.0 / math.sqrt(HD)
    EXP = mybir.ActivationFunctionType.Exp

    with tile.TileContext(nc) as tc, ExitStack() as ctx:
        sb = ctx.enter_context(tc.tile_pool(name="sb", bufs=1))
        xp = ctx.enter_context(tc.tile_pool(name="xp", bufs=2))
        yp = ctx.enter_context(tc.tile_pool(name="yp", bufs=2))
        anp = ctx.enter_context(tc.tile_pool(name="anp", bufs=2))
        lp = ctx.enter_context(tc.tile_pool(name="lp", bufs=2))
        ptp = ctx.enter_context(tc.tile_pool(name="ptp", bufs=2))
        qtp = ctx.enter_context(tc.tile_pool(name="qtp", bufs=1))
        atp = ctx.enter_context(tc.tile_pool(name="atp", bufs=1))
        pp = ctx.enter_context(tc.tile_pool(name="pp", bufs=2, space="PSUM"))
        spp = ctx.enter_context(tc.tile_pool(name="spp", bufs=3, space="PSUM"))
        opp = ctx.enter_context(tc.tile_pool(name="opp", bufs=2, space="PSUM"))
        tpp = ctx.enter_context(tc.tile_pool(name="tpp", bufs=1, space="PSUM"))

        # DMA order matters for startup latency: first compute is K-proj h=0,
        # which needs only bqk + wk + x chunk 0. Halve big transfers so they
        # land on two queues in parallel.
        bqk_t = sb.tile([128, 8], F32)
        nc.sync.dma_start(bqk_t[:], bqk_d)
        # Quarter-split wk/x0 and interleave them so the q-major K projection
        # can start as soon as quarter 0 of both has landed.
        wk_t = sb.tile([128, 8192], FP16)
        xt0 = xp.tile([128, KT16 * CS], FP16, name="xt_t")
        for q4 in range(4):
            nc.sync.dma_start(wk_t[:, q4 * 2048:(q4 + 1) * 2048],
                              wk_d[:, q4 * 2048:(q4 + 1) * 2048])
            nc.sync.dma_start(xt0[:, q4 * 2048:(q4 + 1) * 2048],
                              xt_d[0][:, q4 * 2048:(q4 + 1) * 2048])
        wq_t = sb.tile([128, 8192], FP16)
        for q4 in range(4):
            nc.sync.dma_start(wq_t[:, q4 * 2048:(q4 + 1) * 2048],
                              wq_d[:, q4 * 2048:(q4 + 1) * 2048])
        wv_t = sb.tile([128, 8192], FP16)
        for q4 in range(4):
            nc.sync.dma_start(wv_t[:, q4 * 2048:(q4 + 1) * 2048],
                              wv_d[:, q4 * 2048:(q4 + 1) * 2048])
        wo_t = sb.tile([128, 8192], FP16)
        nc.sync.dma_start(wo_t[:, 0:4096], wo_d[:, 0:4096])
        nc.sync.dma_start(wo_t[:, 4096:8192], wo_d[:, 4096:8192])

        ident = sb.tile([128, 128], FP16)
        masks.make_identity(nc, ident[:])
        # mask01[k, q] = 1.0 if k <= q else 0.0  (keep causal entries)
        mask01 = sb.tile([128, 128], FP16)
        nc.gpsimd.memset(mask01[:], 1.0)
        nc.gpsimd.affine_select(
            out=mask01[:], in_=mask01[:],
            compare_op=mybir.AluOpType.is_ge, fill=0.0,
            base=0, pattern=[[1, 128]], channel_multiplier=-1,
        )

        KT_sb = sb.tile([128, 4 * S], FP16)      # [d, h*S + k_seq]
        V_sb = sb.tile([128, 16 * 516], FP16)    # [k_loc, kb*516 + h*129 + (d|1)]
        V_v4 = V_sb[:].rearrange("p (kb h x) -> p kb h x", kb=16, h=4)
        nc.gpsimd.memset(V_v4[:, :, :, 128:129], 1.0)

        wq_v = wq_t[:].rearrange("p (kt h m) -> p kt h m", kt=KT16, h=4)
        wk_v = wk_t[:].rearrange("p (kt h m) -> p kt h m", kt=KT16, h=4)
        wv_v = wv_t[:].rearrange("p (kt n) -> p kt n", kt=KT16)
        wo_v = wo_t[:].rearrange("p (h j) -> p h j", h=4)

        # Deferred transpose: the last normalize of each pv_norm is transposed
        # during the NEXT phase (scores of h+1, or the Wo matmuls), so PE
        # never idles waiting on the DVE recip->scale chain.
        pend = [None]

        def flush():
            if pend[0] is None:
                return
            at_p, h_p, qs_p, attnT_p = pend[0]
            pend[0] = None
            tt = tpp.tile([128, 128], FP16)
            nc.tensor.transpose(tt[:], at_p[:], ident[:])
            nc.vector.tensor_copy(
                attnT_p[:, h_p * CS + qs_p * 128: h_p * CS + (qs_p + 1) * 128],
                tt[:])

        def scores_exp(c, h, PT, QT):
            # Diagonal-chunk blocks (kl >= 0): queries q < kl*128 are fully
            # masked, so narrow the matmul/exp to columns [kl*128, CS).
            nkb = 4 * (c + 1)
            for kb in range(nkb):
                kl = kb - 4 * c
                off = kl * 128 if kl > 0 else 0
                st = spp.tile([128, CS], F32, name="st")
                nc.tensor.matmul(
                    st[:, off:CS],
                    KT_sb[:, h * S + kb * 128: h * S + (kb + 1) * 128],
                    QT[:, h * CS + off:(h + 1) * CS],
                    start=True, stop=True,
                )
                if kb == 0:
                    flush()
                nc.scalar.activation(PT[:, kb * CS + off:(kb + 1) * CS],
                                     st[:, off:CS], EXP, bias=0.0, scale=SCALE)
                if kl >= 0:
                    d = PT[:, kb * CS + kl * 128: kb * CS + (kl + 1) * 128]
                    nc.vector.tensor_tensor(d, d, mask01[:], mybir.AluOpType.mult)

        def pv_norm(c, h, PT, attnT):
            for qs in range(4):
                qb = 4 * c + qs
                ov = opp.tile([128, 129], F32, name="ov")
                for kb in range(qb + 1):
                    nc.tensor.matmul(
                        ov[:],
                        PT[:, kb * CS + qs * 128: kb * CS + (qs + 1) * 128],
                        V_sb[:, kb * 516 + h * 129: kb * 516 + (h + 1) * 129],
                        start=(kb == 0), stop=(kb == qb),
                    )
                flush()
                linv = lp.tile([128, 1], F32)
                nc.vector.reciprocal(linv[:], ov[:, 128:129])
                at = anp.tile([128, 128], FP16)
                nc.vector.tensor_scalar(at[:], ov[:, 0:128], linv[:], None,
                                        mybir.AluOpType.mult)
                pend[0] = (at, h, qs, attnT)

        # Q/K bias is fused into the PSUM->SBUF eviction (per-partition add
        # on DVE), so no PE bias matmuls. Projections run q-major (quarter of
        # the contraction outer, 4 concurrent PSUM accumulators) so chunk-0
        # compute starts once the first quarter of wk/x has landed.
        # Accumulators 2/3 use the ov ring (not st): projections run while
        # the previous chunk's exp backlog drains, and st would WAR on it.
        def proj_x(ci, c):
            if ci == 0:
                return xt0[:].rearrange("p (kt s) -> p kt s", kt=KT16)
            xt_t = xp.tile([128, KT16 * CS], FP16, name="xt_t")
            nc.sync.dma_start(xt_t[:], xt_d[c])
            return xt_t[:].rearrange("p (kt s) -> p kt s", kt=KT16)

        def proj_K(xv, c):
            kps = ([pp.tile([128, CS], F32, name="pj") for _ in range(2)]
                   + [opp.tile([128, CS], F32, name="ov") for _ in range(2)])
            for q4 in range(4):
                for h in range(4):
                    for kt in range(4 * q4, 4 * q4 + 4):
                        nc.tensor.matmul(kps[h][:], wk_v[:, kt, h], xv[:, kt],
                                         start=(kt == 0), stop=(kt == KT16 - 1))
            for h in range(4):
                nc.vector.tensor_scalar(
                    KT_sb[:, h * S + c * CS: h * S + c * CS + CS], kps[h][:],
                    bqk_t[:, 4 + h:5 + h], None, mybir.AluOpType.add)

        def proj_Q(xv):
            QT = qtp.tile([128, 4 * CS], FP16)
            qps = ([pp.tile([128, CS], F32, name="pj") for _ in range(2)]
                   + [opp.tile([128, CS], F32, name="ov") for _ in range(2)])
            for q4 in range(4):
                for h in range(4):
                    for kt in range(4 * q4, 4 * q4 + 4):
                        nc.tensor.matmul(qps[h][:], wq_v[:, kt, h], xv[:, kt],
                                         start=(kt == 0), stop=(kt == KT16 - 1))
            for h in range(4):
                nc.vector.tensor_scalar(
                    QT[:, h * CS:(h + 1) * CS], qps[h][:],
                    bqk_t[:, h:h + 1], None, mybir.AluOpType.add)
            return QT

        # V bias is applied host-side (bv @ Wo.T added to the output),
        # valid because softmax rows sum to 1.
        def proj_V(xv, c):
            vps = ([pp.tile([128, CS], F32, name="pj") for _ in range(2)]
                   + [opp.tile([128, CS], F32, name="ov") for _ in range(2)])
            for q4 in range(4):
                for sb_i in range(4):
                    for kt in range(4 * q4, 4 * q4 + 4):
                        nc.tensor.matmul(
                            vps[sb_i][:], xv[:, kt, sb_i * 128:(sb_i + 1) * 128],
                            wv_v[:, kt], start=(kt == 0), stop=(kt == KT16 - 1))
            for sb_i in range(4):
                nc.vector.tensor_copy(
                    V_v4[:, 4 * c + sb_i, :, 0:128],
                    vps[sb_i][:].rearrange("p (h d) -> p h d", h=4))

        chunks = [cc for _ in range(repeat) for cc in range(CH)]
        xv = proj_x(0, chunks[0])
        proj_K(xv, chunks[0])
        QT = proj_Q(xv)
        proj_V(xv, chunks[0])
        for ci, c in enumerate(chunks):
            # attention, software-pipelined across heads so ACT exp overlaps
            # PE; the NEXT chunk's projection pieces (pure PE, no ACT) are
            # spread through the h-loop so the exp backlog drains under them.
            # Q-proj must stay after scores(c,3): qtp bufs=1 aliases QT.
            attnT = atp.tile([128, 4 * CS], FP16)   # [d, h*CS + q_loc]
            PTs = [None] * 4
            nxt = ci + 1 < len(chunks)
            for h in range(4):
                PTs[h] = ptp.tile([128, 16 * CS], FP16, name="PT")
                scores_exp(c, h, PTs[h], QT)
                if h >= 1:
                    pv_norm(c, h - 1, PTs[h - 1], attnT)
                if nxt and h == 1:
                    xv_n = proj_x(ci + 1, chunks[ci + 1])
                    proj_K(xv_n, chunks[ci + 1])
                if nxt and h == 2:
                    proj_V(xv_n, chunks[ci + 1])
            if nxt:
                QT = proj_Q(xv_n)
            pv_norm(c, 3, PTs[3], attnT)

            # Wo uses the ov PSUM ring (not pj) so the next chunk's K
            # projection doesn't WAR-stall on the last Wo tiles.
            for qs in range(4):
                ys = yp.tile([128, HID], F32)
                for j in range(4):
                    wp = opp.tile([128, 512], F32, name="ov")
                    for h in range(4):
                        nc.tensor.matmul(
                            wp[:],
                            attnT[:, h * CS + qs * 128: h * CS + (qs + 1) * 128],
                            wo_v[:, h, j * 512:(j + 1) * 512],
                            start=(h == 0), stop=(h == 3),
                        )
                    if qs == 0 and j == 0:
                        flush()
                    nc.vector.tensor_copy(ys[:, j * 512:(j + 1) * 512], wp[:])
                    nc.sync.dma_start(y_d[4 * c + qs][:, j * 512:(j + 1) * 512],
                                      ys[:, j * 512:(j + 1) * 512])

    nc.compile()
    return nc


def _pack_x(xb):
    # xt[c][p, kt*CS + s] = X[c*CS + s, kt*128 + p]
    return np.ascontiguousarray(
        xb.reshape(CH, CS, KT16, 128).transpose(0, 3, 2, 1).reshape(CH, 128, KT16 * CS)
    ).astype(np.float16)


def _pack_wqk(W, hg):
    # w[p, kt*512 + h*128 + m] = W[hg*512 + h*128 + m, kt*128 + p]
    Ws = W[hg * 512:(hg + 1) * 512, :]
    return np.ascontiguousarray(
        Ws.reshape(4, 128, KT16, 128).transpose(2, 3, 0, 1).reshape(128 * KT16, 512)
        .reshape(KT16, 128, 512).transpose(1, 0, 2).reshape(128, 8192)
    ).astype(np.float16)


def _pack_wv(W, hg):
    # w[p, kt*512 + n] = W[hg*512 + n, kt*128 + p]
    Ws = W[hg * 512:(hg + 1) * 512, :]
    return np.ascontiguousarray(
        Ws.reshape(512, KT16, 128).transpose(2, 1, 0).reshape(128, 8192)
    ).astype(np.float16)


def _pack_bqk(bq, bk, hg):
    # bqk[m, h] = bq[hg*512 + h*128 + m]; bqk[m, 4+h] = bk[...]
    qs = bq[hg * 512:(hg + 1) * 512].reshape(4, 128)
    ks = bk[hg * 512:(hg + 1) * 512].reshape(4, 128)
    return np.ascontiguousarray(
        np.concatenate([qs.T, ks.T], axis=1)).astype(np.float32)


def _pack_wo(W, hg):
    # w[p=d, h*2048 + j] = W[j, hg*512 + h*128 + d]
    Ws = W[:, hg * 512:(hg + 1) * 512]
    return np.ascontiguousarray(
        Ws.reshape(HID, 4, 128).transpose(2, 1, 0).reshape(128, 8192)
    ).astype(np.float16)


def kernel(hidden_states, Wq, bq, Wk, bk, Wv, bv, Wo, bo):
    global _NC, _RUNNER, LAST_DEVICE_NS
    if _NC is None:
        _NC = build_program()
        _RUNNER = _Runner(_NC)

    hs = np.asarray(hidden_states, dtype=np.float32)
    Wq = np.asarray(Wq, dtype=np.float32)
    Wk = np.asarray(Wk, dtype=np.float32)
    Wv = np.asarray(Wv, dtype=np.float32)
    Wo = np.asarray(Wo, dtype=np.float32)
    bq = np.asarray(bq, dtype=np.float32)
    bk = np.asarray(bk, dtype=np.float32)
    bv = np.asarray(bv, dtype=np.float32)
    bo = np.asarray(bo, dtype=np.float32)

    xpacks = [_pack_x(hs[b]) for b in range(B)]
    wpacks = []
    for hg in range(4):
        wpacks.append({
            "wq": _pack_wqk(Wq, hg),
            "wk": _pack_wqk(Wk, hg),
            "wv": _pack_wv(Wv, hg),
            "wo": _pack_wo(Wo, hg),
            "bqk": _pack_bqk(bq, bk, hg),
        })

    in_maps = []
    for core in range(8):
        b, hg = divmod(core, 4)
        m = dict(wpacks[hg])
        m["xt"] = xpacks[b]
        in_maps.append(m)

    results, LAST_DEVICE_NS = _RUNNER(in_maps)

    out = np.zeros((B, S, HID), np.float64)
    for core in range(8):
        b, hg = divmod(core, 4)
        out[b] += results[core]["y"].reshape(S, HID).astype(np.float64)
    # bv commutes through softmax (rows sum to 1): context = A @ Vraw + bv,
    # so y gains bv @ Wo.T, applied here in fp64 alongside bo.
    out += bo.astype(np.float64) + bv.astype(np.float64) @ Wo.astype(np.float64).T
    return out.astype(np.float32)
